# revision 1
# baseline (speedup 1.0000x reference)
"""Trainium2 Bass kernel for nn_DecoderA (neural BP / GNN message passing decoder).

Strategy: pure data parallel over batch (128 items -> 8 cores x 16 items).
Per core, items are processed in 4 groups of 4; each group's message state
M [4*288, 576] lives in SBUF as 9 tiles of [128, 576] ((b,m)-rows x n) for
all 5 BP iterations.  Per iteration, per tile j (paired for wide tail ops):

  PSUM   vr   = Esel@A - I@M - 40          (PE matmuls; A = x_t + sum_m M)
  DVE    vm   = (vr min -25) * mask        (scalar_tensor_tensor from PSUM)
  DVE    v2   = vm max -55                 (completes clip(V,+-15)-40, off-mask 0)
  ACT    te   = tanh(0.5*v2 + 20)          (== masked tanh(clip(V)/2), 1 off-mask)
  DVE    P    = prefix-product scan of te  (last col = row product)
  ACT    a1   = |te + P|                   (Abs with bias=P)
  DVE    a2   = max(|te - P|, 1e-38)       (tensor_scalar two-op)
  ACT    l1, l2 = ln(a + 1e-38)            (pair-wide, in place)
  GPS    lq   = l1 - l2                    (== 2*atanh(P/te), div-free, pair-wide)
  DVE    lqc  = clip(lq, +-2*atanh(1-1e-6))
  GPS    d    = lqc * Wg                   (Wg = gate*w_cv*H, from host)
  DVE    M    = M*(1-gate) + d             (pair-wide stt)
  PE     post = Esel^T @ M                 (per-item column sums, accumulated)

Host does the cheap pre/post work (LLR normalization, pooling, sigmoid).
"""

import sys

import numpy as np

sys.path.insert(0, "/opt/trn_rl_repo")

import concourse.bacc as bacc  # noqa: E402
import concourse.tile as tile  # noqa: E402
from concourse import mybir  # noqa: E402
from concourse.bass_utils import run_bass_kernel_spmd  # noqa: E402

F32 = mybir.dt.float32
BF16 = mybir.dt.bfloat16
ALU = mybir.AluOpType
ACT = mybir.ActivationFunctionType

B = 128
MCHK = 288
NVAR = 576
KINFO = 288
T = 5
NCORES = 8
BL = B // NCORES          # 16 items per core
GI = 4                    # items per group
NG = BL // GI             # 4 groups
NT = GI * MCHK // 128     # 9 tiles of [128, NVAR] per group
HC = NVAR // 2            # 288, matmul N-chunk (<=512 per PSUM bank)

_CLIP_C = float(2.0 * np.arctanh(np.float64(np.float32(1.0 - 1e-6))))


def _build(gate: float):
    nc = bacc.Bacc("TRN2", target_bir_lowering=False, debug=False)

    wg_d = nc.dram_tensor("wg", [BL * MCHK, NVAR], F32, kind="ExternalInput").ap()
    xs_d = nc.dram_tensor("xs", [BL, T * NVAR], F32, kind="ExternalInput").ap()
    esel_d = nc.dram_tensor("esel", [128, NT * GI], F32, kind="ExternalInput").ap()
    eselt_d = nc.dram_tensor("eselt", [GI, NT * 128], F32, kind="ExternalInput").ap()
    negi_d = nc.dram_tensor("negi", [128, 128], F32, kind="ExternalInput").ap()
    posts_d = nc.dram_tensor("posts", [BL, T * NVAR], F32, kind="ExternalOutput").ap()

    one_m_g = float(1.0 - gate)

    with tile.TileContext(nc) as tc:
        with (
            tc.tile_pool(name="consts", bufs=1) as consts,
            tc.tile_pool(name="wg", bufs=2) as wg_pool,
            tc.tile_pool(name="mask", bufs=2) as mask_pool,
            tc.tile_pool(name="mstate", bufs=2) as m_pool,
            tc.tile_pool(name="atile", bufs=3) as a_pool,
            tc.tile_pool(name="t1", bufs=2) as t1_pool,
            tc.tile_pool(name="t2", bufs=2) as t2_pool,
            tc.tile_pool(name="t3", bufs=2) as t3_pool,
            tc.tile_pool(name="pp", bufs=6) as pp_pool,
            tc.tile_pool(name="psum_v", bufs=2, space="PSUM") as psv_pool,
            tc.tile_pool(name="psum_post", bufs=2, space="PSUM") as psp_pool,
        ):
            esel = consts.tile([128, NT, GI], F32)
            nc.sync.dma_start(out=esel, in_=esel_d.rearrange("p (j g) -> p j g", g=GI))
            eselt = consts.tile([GI, NT, 128], F32)
            nc.sync.dma_start(
                out=eselt, in_=eselt_d.rearrange("g (j p) -> g j p", p=128)
            )
            negi = consts.tile([128, 128], F32)
            nc.sync.dma_start(out=negi, in_=negi_d)
            ones1 = consts.tile([1, 128], F32)
            nc.vector.memset(ones1, 1.0)
            neg40 = consts.tile([1, HC], F32)
            nc.vector.memset(neg40, -40.0)
            b20 = consts.tile([128, 1], F32)
            nc.vector.memset(b20, 20.0)
            b38 = consts.tile([128, 1], F32)
            nc.vector.memset(b38, 1e-38)
            xsall = consts.tile([128, T, 2, HC], F32)
            for g in range(NG):
                nc.sync.dma_start(
                    out=xsall[32 * g : 32 * g + GI],
                    in_=xs_d[g * GI : (g + 1) * GI].rearrange(
                        "b (t c n) -> b t c n", t=T, c=2
                    ),
                )
            postsall = consts.tile([128, T, 2, HC], F32)

            # tile pairs: (0,1) (2,3) (4,5) (6,7) (8,)
            pairs = [(0, 1), (2, 3), (4, 5), (6, 7), (8,)]

            for g in range(NG):
                # ---- group loads ----
                wg_g = wg_pool.tile([128, NT, NVAR], F32)
                nc.sync.dma_start(
                    out=wg_g,
                    in_=wg_d[g * NT * 128 : (g + 1) * NT * 128, :].rearrange(
                        "(j p) n -> p j n", p=128
                    ),
                )
                mask_g = mask_pool.tile([128, NT, NVAR], BF16)
                nc.vector.tensor_scalar(
                    out=mask_g, in0=wg_g, scalar1=0.0, scalar2=None, op0=ALU.not_equal
                )
                xs_g = xsall[32 * g : 32 * g + GI]
                m_g = m_pool.tile([128, NT, NVAR], F32)

                a_cur = a_pool.tile([GI, 2, HC], F32, tag="a_cur", name="a_cur")
                nc.vector.tensor_copy(a_cur, xs_g[:, 0])
                for t in range(T):
                    post_ps = psp_pool.tile([GI, 2, 512], F32)
                    for pi, pj in enumerate(pairs):
                        w = len(pj)
                        j0 = pj[0]
                        # pair-wide staging tiles (singleton uses first half)
                        v2p = t1_pool.tile([128, 2, NVAR], F32, tag="v2", name="v2")[:, :w]
                        a12 = t2_pool.tile([128, 2, 2, NVAR], F32, tag="a12", name="a12")
                        tep = t1_pool.tile([128, 2, NVAR], F32, tag="te", name="te")[:, :w]
                        for jj, j in enumerate(pj):
                            rhs_a = a_cur
                            mj = m_g[:, j]
                            # vr = Esel@A - I@M - 40  (PSUM)
                            v_ps = psv_pool.tile([128, 2, 512], F32)
                            for c in range(2):
                                nc.tensor.matmul(
                                    v_ps[:, c, :HC],
                                    eselt[:, j],
                                    rhs_a[:, c],
                                    start=True,
                                    stop=False,
                                )
                                if t > 0:
                                    nc.tensor.matmul(
                                        v_ps[:, c, :HC],
                                        negi,
                                        mj[:, c * HC : (c + 1) * HC],
                                        start=False,
                                        stop=False,
                                    )
                                nc.tensor.matmul(
                                    v_ps[:, c, :HC],
                                    ones1,
                                    neg40,
                                    start=False,
                                    stop=True,
                                )
                            # vm = (vr min -25) * mask   (upper clip + mask)
                            nc.vector.scalar_tensor_tensor(
                                out=v2p[:, jj].rearrange("p (c n) -> p c n", c=2),
                                in0=v_ps[:, :, :HC],
                                scalar=-25.0,
                                in1=mask_g[:, j].rearrange("p (c n) -> p c n", c=2),
                                op0=ALU.min,
                                op1=ALU.mult,
                            )
                            # v2 = vm max -55   (lower clip; off-mask stays 0)
                            nc.vector.tensor_scalar(
                                out=v2p[:, jj], in0=v2p[:, jj], scalar1=-55.0,
                                scalar2=None, op0=ALU.max,
                            )
                            # te = tanh(0.5*v2 + 20)
                            nc.scalar.activation(
                                tep[:, jj], v2p[:, jj], ACT.Tanh, bias=b20, scale=0.5
                            )
                            # P = prod_n te (prefix-product scan into the dead
                            # v2 half; last col = P)
                            pscan = v2p[:, jj]
                            nc.vector.tensor_tensor_scan(
                                out=pscan, data0=tep[:, jj], data1=tep[:, jj],
                                initial=1.0, op0=ALU.mult, op1=ALU.bypass,
                            )
                            p_t = pscan[:, NVAR - 1 : NVAR]
                            # a1 = |te + P|, a2 = |P - te|  (ACT Abs, bias=P)
                            nc.scalar.activation(
                                a12[:, 0, jj], tep[:, jj], ACT.Abs, bias=p_t, scale=1.0
                            )
                            nc.scalar.activation(
                                a12[:, 1, jj], tep[:, jj], ACT.Abs, bias=p_t, scale=-1.0
                            )
                        # ---- pair-wide tail ----
                        af = a12[:, :, :w]
                        nc.scalar.activation(af, af, ACT.Ln, bias=b38)
                        a1f = a12[:, 0, :w]
                        a2f = a12[:, 1, :w]
                        lqf = t3_pool.tile([128, 2, NVAR], F32, tag="lq", name="lq")[:, :w]
                        nc.gpsimd.tensor_tensor(out=lqf, in0=a1f, in1=a2f,
                                                op=ALU.subtract)
                        nc.vector.tensor_scalar(
                            out=lqf, in0=lqf, scalar1=_CLIP_C, scalar2=-_CLIP_C,
                            op0=ALU.min, op1=ALU.max,
                        )
                        mpair = m_g[:, j0 : j0 + w]
                        wgp = wg_g[:, j0 : j0 + w]
                        if t == 0:
                            nc.gpsimd.tensor_tensor(out=mpair, in0=lqf, in1=wgp,
                                                    op=ALU.mult)
                        else:
                            nc.gpsimd.tensor_tensor(out=lqf, in0=lqf, in1=wgp,
                                                    op=ALU.mult)
                            nc.vector.scalar_tensor_tensor(
                                out=mpair, in0=mpair, scalar=one_m_g, in1=lqf,
                                op0=ALU.mult, op1=ALU.add,
                            )
                        # posterior accumulation: post += Esel_j^T @ M_j
                        for jj, j in enumerate(pj):
                            for c in range(2):
                                nc.tensor.matmul(
                                    post_ps[:, c, :HC],
                                    esel[:, j],
                                    m_g[:, j, c * HC : (c + 1) * HC],
                                    start=(j == 0),
                                    stop=(j == NT - 1),
                                )
                    # posts_raw[g, :, t, :] = post (host adds x_t)
                    nc.scalar.copy(postsall[32 * g : 32 * g + GI, t],
                                   post_ps[:, :, :HC])
                    if t + 1 < T:
                        a_cur = a_pool.tile([GI, 2, HC], F32, tag="a_cur",
                                            name="a_cur")
                        nc.vector.tensor_add(a_cur, post_ps[:, :, :HC], xs_g[:, t + 1])
            for g in range(NG):
                nc.sync.dma_start(
                    out=posts_d[g * GI : (g + 1) * GI].rearrange(
                        "b (t c n) -> b t c n", t=T, c=2
                    ),
                    in_=postsall[32 * g : 32 * g + GI],
                )
    nc.compile()
    return nc


_CACHE = {}


def _get_nc(gate: float):
    key = round(gate, 12)
    if key not in _CACHE:
        _CACHE[key] = _build(gate)
    return _CACHE[key]


def _host_prep(inputs, H, sigma2, input_ponderation, w_cv, gate_logit):
    f32 = np.float32
    gate = float(1.0 / (1.0 + np.exp(-np.float64(gate_logit))))

    llrs = (f32(-4.0) * inputs / sigma2).astype(f32)
    norm_llrs = llrs / np.mean(np.abs(llrs), axis=-1, keepdims=True, dtype=f32)
    xs = (norm_llrs[:, None, :] * input_ponderation[None, :, :]).astype(f32)  # [B,T,N]

    wg_full = (f32(gate) * w_cv[None, :, :] * H.astype(f32)).astype(f32)  # [B,M,N]

    # selector constants (same for every core)
    rows = np.arange(GI * MCHK)
    esel = np.zeros((128, NT, GI), f32)
    eselt = np.zeros((GI, NT, 128), f32)
    for j in range(NT):
        for p in range(128):
            k = int(rows[j * 128 + p] // MCHK)
            esel[p, j, k] = 1.0
            eselt[k, j, p] = 1.0
    negi = (-np.eye(128, dtype=f32))

    in_maps = []
    for c in range(NCORES):
        sl = slice(c * BL, (c + 1) * BL)
        in_maps.append(
            {
                "wg": np.ascontiguousarray(wg_full[sl].reshape(BL * MCHK, NVAR)),
                "xs": np.ascontiguousarray(xs[sl].reshape(BL, T * NVAR)),
                "esel": np.ascontiguousarray(esel.reshape(128, NT * GI)),
                "eselt": np.ascontiguousarray(eselt.reshape(GI, NT * 128)),
                "negi": negi,
            }
        )
    return gate, norm_llrs, xs, in_maps


def _host_post(posts_raw, xs, norm_llrs, out_ponderation, skip_ponderation):
    f32 = np.float32
    posts = (posts_raw + xs).astype(f32)  # add x_t back in
    norm_out = posts / np.mean(np.abs(posts), axis=-1, keepdims=True, dtype=f32)
    pooled = np.mean(out_ponderation[None] * norm_out, axis=-2, dtype=f32)
    out = (pooled + skip_ponderation * norm_llrs).astype(f32)
    return (1.0 / (1.0 + np.exp(out[:, :KINFO], dtype=f32))).astype(f32)


def run(trace=False, **inputs):
    inputs = {k: np.asarray(v) for k, v in inputs.items()}
    gate, norm_llrs, xs, in_maps = _host_prep(
        inputs["inputs"],
        inputs["H"],
        inputs["sigma2"],
        inputs["input_ponderation"],
        inputs["w_cv"],
        inputs["gate_logit"],
    )
    nc = _get_nc(gate)
    res = run_bass_kernel_spmd(
        nc, in_maps, core_ids=list(range(NCORES)), trace=trace
    )
    posts_raw = np.concatenate(
        [r["posts"].reshape(BL, T, NVAR) for r in res.results], axis=0
    )
    out = _host_post(
        posts_raw, xs, norm_llrs,
        inputs["out_ponderation"], inputs["skip_ponderation"],
    )
    return out, res


def kernel(**inputs) -> np.ndarray:
    out, _ = run(trace=False, **inputs)
    return out



# revision 18
# speedup vs baseline: 1.6877x; 1.6877x over previous
"""Trainium2 Bass kernel for nn_DecoderA (neural BP / GNN message passing decoder).

Strategy: pure data parallel over batch (128 items -> 8 cores x 16 items).
Per core, items are processed in 4 groups of 4; each group's message state
M [4*288, 576] lives in SBUF as 9 tiles of [128, 576] ((b,m)-rows x n) for
all 5 BP iterations.  Per iteration, per tile j:

  PE    psumA = Esel@A           (fp32r matmul; A = x_t + sum_m M_prev)
  DVE   v     = psumA - M        (raw V; off-edge lanes carry garbage)
  ACT   te    = tanh(0.5 v)      (pair-wide; no clip - tanh saturates)
  DVE   P     = ttreduce(max(te, offm), mult)   (off-edge fixed to +1 only
                                                 inside the row product)
  ACT   a1    = |te + P|         (Abs, bias=P)   -> bf16
  DVE   a2    = |te - P|         (ts: sub P, abs_max 0) -> bf16
  ACT   l     = ln(a + 1e-6)     (batch-wide; guard == clip at +-14.5)
  GPS   lq    = l1 - l2          (== 2*atanh(P/te), div-free)
  DVE   d     = lq * wg          (wg = gate*w_cv*H bf16; zero off-edge)
  GPS   M     = 0.5*M + d        (damped update; t=0 writes d directly)
  PE    post += Esel^T @ M       (fp32r, accumulated over tiles)

Off-edge lanes are never masked: wg=0 kills them in d, and the +-1 offm
tensor fixes them inside the product reduce.  The ln guard 1e-6 doubles as
the reference's clip(2 atanh(r), +-14.5) (ln(2/1e-6) = 14.5).
Host does the cheap pre/post work (LLR normalization, pooling, sigmoid).
"""

import sys

import numpy as np

sys.path.insert(0, "/opt/trn_rl_repo")

import ml_dtypes  # noqa: E402

import concourse.bacc as bacc  # noqa: E402
import concourse.tile as tile  # noqa: E402
from concourse import mybir  # noqa: E402
from concourse.bass_utils import run_bass_kernel_spmd  # noqa: E402

F32 = mybir.dt.float32
F32R = mybir.dt.float32r
BF16 = mybir.dt.bfloat16
F16 = mybir.dt.float16


def _f16(ap):
    return ap.bitcast(mybir.dt.float16)
ALU = mybir.AluOpType
ACT = mybir.ActivationFunctionType

B = 128
MCHK = 288
NVAR = 576
KINFO = 288
T = 5
NCORES = 8
BL = B // NCORES          # 16 items per core
GI = 4                    # items per group
NG = BL // GI             # 4 groups
NT = GI * MCHK // 128     # 9 tiles of [128, NVAR] per group
HC = NVAR // 2            # 288, matmul N-chunk (<=512 per PSUM bank)

_GUARD = 1e-30            # ln guard: avoids ln(0) when te == P == 0
_CLIP_C = float(2.0 * np.arctanh(np.float64(np.float32(1.0 - 1e-6))))
_TE_CAP = float(np.float32(np.tanh(np.float64(np.float32(7.5)))))


def _build(gate: float):
    nc = bacc.Bacc("TRN2", target_bir_lowering=False, debug=False)

    wg_d = nc.dram_tensor("wg", [BL * MCHK, NVAR], F16, kind="ExternalInput").ap()
    offm_d = nc.dram_tensor("offm", [BL * MCHK, NVAR], BF16,
                            kind="ExternalInput").ap()
    xs_d = nc.dram_tensor("xs", [BL, T * NVAR], F32, kind="ExternalInput").ap()
    esel_d = nc.dram_tensor("esel", [128, NT * GI], F16, kind="ExternalInput").ap()
    eselt_d = nc.dram_tensor("eselt", [GI, NT * 128], F32R,
                             kind="ExternalInput").ap()
    posts_d = nc.dram_tensor("posts", [BL, T * NVAR], F32, kind="ExternalOutput").ap()

    one_m_g = float(1.0 - gate)

    with tile.TileContext(nc) as tc:
        with (
            tc.tile_pool(name="consts", bufs=1) as consts,
            tc.tile_pool(name="wg", bufs=2) as wg_pool,
            tc.tile_pool(name="offm", bufs=2) as offm_pool,
            tc.tile_pool(name="mstate", bufs=2) as m_pool,
            tc.tile_pool(name="atile", bufs=3) as a_pool,
            tc.tile_pool(name="vte", bufs=3) as vte_pool,
            tc.tile_pool(name="a12", bufs=2) as a12_pool,
            tc.tile_pool(name="pprod", bufs=2) as p_pool,
            tc.tile_pool(name="psum_v", bufs=2, space="PSUM") as psv_pool,
            tc.tile_pool(name="psum_post", bufs=2, space="PSUM") as psp_pool,
        ):
            esel = consts.tile([128, NT, GI], F16)
            nc.sync.dma_start(out=esel, in_=esel_d.rearrange("p (j g) -> p j g", g=GI))
            eselt = consts.tile([GI, NT, 128], F32R)
            nc.sync.dma_start(
                out=eselt, in_=eselt_d.rearrange("g (j p) -> g j p", p=128)
            )
            b_guard = consts.tile([128, 1], F32)
            nc.vector.memset(b_guard, _GUARD)
            xsall = consts.tile([128, T, 2, HC], F32)
            for g in range(NG):
                nc.sync.dma_start(
                    out=xsall[32 * g : 32 * g + GI],
                    in_=xs_d[g * GI : (g + 1) * GI].rearrange(
                        "b (t c n) -> b t c n", t=T, c=2
                    ),
                )
            postsall = consts.tile([128, T, 2, HC], F32)

            # tile pairs: (0,1) (2,3) (4,5) (6,7) (8,)
            pairs = [(0, 1), (2, 3), (4, 5), (6, 7), (8,)]

            for g in range(NG):
                # ---- group loads ----
                wg_g = wg_pool.tile([128, NT, NVAR], F16)
                nc.sync.dma_start(
                    out=wg_g,
                    in_=wg_d[g * NT * 128 : (g + 1) * NT * 128, :].rearrange(
                        "(j p) n -> p j n", p=128
                    ),
                )
                offm_g = offm_pool.tile([128, NT, NVAR], BF16)
                nc.sync.dma_start(
                    out=offm_g,
                    in_=offm_d[g * NT * 128 : (g + 1) * NT * 128, :].rearrange(
                        "(j p) n -> p j n", p=128
                    ),
                )
                xs_g = xsall[32 * g : 32 * g + GI]
                m_g = m_pool.tile([128, NT, NVAR], F32)

                a_cur = a_pool.tile([GI, 2, HC], F32R, tag="a_cur", name="a_cur")
                nc.vector.tensor_copy(a_cur, xs_g[:, 0])
                for t in range(T):
                    post_ps = psp_pool.tile([GI, 2, 512], F32)
                    ptile = p_pool.tile([128, NT], F32, tag="pp", name="pp")
                    a12 = a12_pool.tile([128, NT, 2, NVAR], BF16, tag="a12",
                                        name="a12")
                    vtes = {}
                    # ---- stage A: broadcast A, subtract M  (PE + DVE) ----
                    for pi, pj in enumerate(pairs):
                        w = len(pj)
                        vte = vte_pool.tile([128, 2, NVAR], F32, tag="vte",
                                            name="vte")[:, :w]
                        vtes[pi] = vte
                        for jj, j in enumerate(pj):
                            v_ps = psv_pool.tile([128, 2, 512], F32)
                            for c in range(2):
                                nc.tensor.matmul(
                                    v_ps[:, c, :HC],
                                    eselt[:, j],
                                    a_cur[:, c],
                                    start=True,
                                    stop=True,
                                )
                            if t == 0:
                                # M = 0: v is just the broadcast A
                                nc.vector.tensor_copy(
                                    vte[:, jj].rearrange("p (c n) -> p c n", c=2),
                                    v_ps[:, :, :HC],
                                )
                            else:
                                nc.vector.tensor_tensor(
                                    out=vte[:, jj].rearrange("p (c n) -> p c n", c=2),
                                    in0=v_ps[:, :, :HC],
                                    in1=m_g[:, j].rearrange("p (c n) -> p c n", c=2),
                                    op=ALU.subtract,
                                )
                    # ---- stage B: tanh (ACT, pair-wide) ----
                    for pi, pj in enumerate(pairs):
                        nc.scalar.activation(vtes[pi], vtes[pi], ACT.Tanh,
                                             bias=0.0, scale=0.5)
                    # ---- stage B2: cap te to tanh(7.5) == reference's V clip ----
                    for pi, pj in enumerate(pairs):
                        nc.vector.tensor_scalar(
                            out=vtes[pi], in0=vtes[pi], scalar1=_TE_CAP,
                            scalar2=-_TE_CAP, op0=ALU.min, op1=ALU.max,
                        )
                    # ---- stage C: row product with off-edge fix ----
                    for pi, pj in enumerate(pairs):
                        w = len(pj)
                        te_p = vtes[pi]
                        nc.vector.tensor_tensor(
                            out=te_p, in0=te_p,
                            in1=offm_g[:, pj[0] : pj[0] + w],
                            op=ALU.max,
                        )
                        for jj, j in enumerate(pj):
                            nc.vector.tensor_reduce(
                                out=ptile[:, j : j + 1],
                                in_=vtes[pi][:, jj],
                                axis=mybir.AxisListType.X,
                                op=ALU.mult,
                            )
                    # ---- stage D: |te +- P|  (ACT abs / DVE ts) ----
                    for pi, pj in enumerate(pairs):
                        for jj, j in enumerate(pj):
                            te = vtes[pi][:, jj]
                            p_t = ptile[:, j : j + 1]
                            nc.scalar.activation(a12[:, j, 0], te, ACT.Abs,
                                                 bias=p_t, scale=1.0)
                            nc.scalar.activation(a12[:, j, 1], te, ACT.Abs,
                                                 bias=p_t, scale=-1.0)
                    # ---- stage E: ln (ACT, batched over 4 tiles) ----
                    for j0 in (0, 4, 8):
                        jw = min(4, NT - j0)
                        nc.scalar.activation(
                            _f16(a12[:, j0 : j0 + jw]), a12[:, j0 : j0 + jw],
                            ACT.Ln, bias=b_guard,
                        )
                    # ---- stage F..H: lq, d, M update (GPS + DVE) ----
                    for pi, pj in enumerate(pairs):
                        w = len(pj)
                        j0 = pj[0]
                        l1 = _f16(a12[:, j0 : j0 + w, 0])
                        l2 = _f16(a12[:, j0 : j0 + w, 1])
                        nc.gpsimd.tensor_tensor(out=l1, in0=l1, in1=l2,
                                                op=ALU.subtract)
                        nc.vector.tensor_scalar(
                            out=l1, in0=l1, scalar1=_CLIP_C, scalar2=-_CLIP_C,
                            op0=ALU.min, op1=ALU.max,
                        )
                        mpair = m_g[:, j0 : j0 + w]
                        wgp = wg_g[:, j0 : j0 + w]
                        d = _f16(a12[:, j0 : j0 + w, 1])
                        nc.vector.tensor_tensor(out=d, in0=l1, in1=wgp,
                                                op=ALU.mult)
                        if t == 0:
                            nc.vector.tensor_copy(mpair, d)
                        else:
                            nc.vector.scalar_tensor_tensor(
                                out=mpair, in0=mpair, scalar=one_m_g, in1=d,
                                op0=ALU.mult, op1=ALU.add,
                            )
                        # posterior increment: post += Esel_j^T @ d_j  (bf16)
                        for jj, j in enumerate(pj):
                            for c in range(2):
                                nc.tensor.matmul(
                                    post_ps[:, c, :HC],
                                    esel[:, j],
                                    _f16(a12[:, j, 1, c * HC : (c + 1) * HC]),
                                    start=(j == 0),
                                    stop=(j == NT - 1),
                                )
                    # posts(t) = (1-gate)*posts(t-1) + sum_m d  (host adds x_t)
                    posts_t = postsall[32 * g : 32 * g + GI, t]
                    if t == 0:
                        nc.scalar.copy(posts_t, post_ps[:, :, :HC])
                    else:
                        nc.vector.scalar_tensor_tensor(
                            out=posts_t,
                            in0=postsall[32 * g : 32 * g + GI, t - 1],
                            scalar=one_m_g,
                            in1=post_ps[:, :, :HC],
                            op0=ALU.mult,
                            op1=ALU.add,
                        )
                    if t + 1 < T:
                        a_cur = a_pool.tile([GI, 2, HC], F32R, tag="a_cur",
                                            name="a_cur")
                        nc.vector.tensor_add(a_cur, posts_t, xs_g[:, t + 1])
            for g in range(NG):
                nc.sync.dma_start(
                    out=posts_d[g * GI : (g + 1) * GI].rearrange(
                        "b (t c n) -> b t c n", t=T, c=2
                    ),
                    in_=postsall[32 * g : 32 * g + GI],
                )
    nc.compile()
    return nc


_CACHE = {}


def _get_nc(gate: float):
    key = round(gate, 12)
    if key not in _CACHE:
        _CACHE[key] = _build(gate)
    return _CACHE[key]


def _host_prep(inputs, H, sigma2, input_ponderation, w_cv, gate_logit):
    f32 = np.float32
    gate = float(1.0 / (1.0 + np.exp(-np.float64(gate_logit))))

    llrs = (f32(-4.0) * inputs / sigma2).astype(f32)
    norm_llrs = llrs / np.mean(np.abs(llrs), axis=-1, keepdims=True, dtype=f32)
    xs = (norm_llrs[:, None, :] * input_ponderation[None, :, :]).astype(f32)  # [B,T,N]

    Hf = H.astype(f32)
    wg_full = (f32(gate) * w_cv[None, :, :] * Hf).astype(np.float16)
    offm_full = (f32(1.0) - f32(2.0) * Hf).astype(ml_dtypes.bfloat16)  # +1 off, -1 on

    # selector constants (same for every core)
    rows = np.arange(GI * MCHK)
    esel = np.zeros((128, NT, GI), f32)
    eselt = np.zeros((GI, NT, 128), f32)
    for j in range(NT):
        for p in range(128):
            k = int(rows[j * 128 + p] // MCHK)
            esel[p, j, k] = 1.0
            eselt[k, j, p] = 1.0

    in_maps = []
    for c in range(NCORES):
        sl = slice(c * BL, (c + 1) * BL)
        in_maps.append(
            {
                "wg": np.ascontiguousarray(wg_full[sl].reshape(BL * MCHK, NVAR)),
                "offm": np.ascontiguousarray(offm_full[sl].reshape(BL * MCHK, NVAR)),
                "xs": np.ascontiguousarray(xs[sl].reshape(BL, T * NVAR)),
                "esel": np.ascontiguousarray(
                    esel.reshape(128, NT * GI).astype(np.float16)
                ),
                "eselt": np.ascontiguousarray(eselt.reshape(GI, NT * 128)),
            }
        )
    return gate, norm_llrs, xs, in_maps


def _host_post(posts_raw, xs, norm_llrs, out_ponderation, skip_ponderation):
    f32 = np.float32
    posts = (posts_raw + xs).astype(f32)  # add x_t back in
    norm_out = posts / np.mean(np.abs(posts), axis=-1, keepdims=True, dtype=f32)
    pooled = np.mean(out_ponderation[None] * norm_out, axis=-2, dtype=f32)
    out = (pooled + skip_ponderation * norm_llrs).astype(f32)
    return (1.0 / (1.0 + np.exp(out[:, :KINFO], dtype=f32))).astype(f32)


def run(trace=False, **inputs):
    inputs = {k: np.asarray(v) for k, v in inputs.items()}
    gate, norm_llrs, xs, in_maps = _host_prep(
        inputs["inputs"],
        inputs["H"],
        inputs["sigma2"],
        inputs["input_ponderation"],
        inputs["w_cv"],
        inputs["gate_logit"],
    )
    nc = _get_nc(gate)
    res = run_bass_kernel_spmd(
        nc, in_maps, core_ids=list(range(NCORES)), trace=trace
    )
    posts_raw = np.concatenate(
        [r["posts"].reshape(BL, T, NVAR) for r in res.results], axis=0
    )
    out = _host_post(
        posts_raw, xs, norm_llrs,
        inputs["out_ponderation"], inputs["skip_ponderation"],
    )
    return out, res


def kernel(**inputs) -> np.ndarray:
    out, _ = run(trace=False, **inputs)
    return out


# revision 20
# speedup vs baseline: 1.7372x; 1.0294x over previous
"""Trainium2 Bass kernel for nn_DecoderA (neural BP / GNN message passing decoder).

Strategy: pure data parallel over batch (128 items -> 8 cores x 16 items).
Per core, items are processed in 4 groups of 4; each group's message state
M [4*288, 576] lives in SBUF as 9 tiles of [128, 576] ((b,m)-rows x n) for
all 5 BP iterations.  Per iteration, per tile j:

  PE    psumA = Esel@A           (fp32r matmul; A = x_t + sum_m M_prev)
  DVE   v     = psumA - M        (raw V; off-edge lanes carry garbage)
  ACT   te    = tanh(0.5 v)      (pair-wide; no clip - tanh saturates)
  DVE   P     = ttreduce(max(te, offm), mult)   (off-edge fixed to +1 only
                                                 inside the row product)
  ACT   a1    = |te + P|         (Abs, bias=P)   -> bf16
  DVE   a2    = |te - P|         (ts: sub P, abs_max 0) -> bf16
  ACT   l     = ln(a + 1e-6)     (batch-wide; guard == clip at +-14.5)
  GPS   lq    = l1 - l2          (== 2*atanh(P/te), div-free)
  DVE   d     = lq * wg          (wg = gate*w_cv*H bf16; zero off-edge)
  GPS   M     = 0.5*M + d        (damped update; t=0 writes d directly)
  PE    post += Esel^T @ M       (fp32r, accumulated over tiles)

Off-edge lanes are never masked: wg=0 kills them in d, and the +-1 offm
tensor fixes them inside the product reduce.  The ln guard 1e-6 doubles as
the reference's clip(2 atanh(r), +-14.5) (ln(2/1e-6) = 14.5).
Host does the cheap pre/post work (LLR normalization, pooling, sigmoid).
"""

import sys

import numpy as np

sys.path.insert(0, "/opt/trn_rl_repo")

import ml_dtypes  # noqa: E402

import concourse.bacc as bacc  # noqa: E402
import concourse.tile as tile  # noqa: E402
from concourse import mybir  # noqa: E402
from concourse.bass_utils import run_bass_kernel_spmd  # noqa: E402

F32 = mybir.dt.float32
F32R = mybir.dt.float32r
BF16 = mybir.dt.bfloat16
F16 = mybir.dt.float16


def _f16(ap):
    return ap.bitcast(mybir.dt.float16)
ALU = mybir.AluOpType
ACT = mybir.ActivationFunctionType

B = 128
MCHK = 288
NVAR = 576
KINFO = 288
T = 5
NCORES = 8
BL = B // NCORES          # 16 items per core
GI = 4                    # items per group
NG = BL // GI             # 4 groups
NT = GI * MCHK // 128     # 9 tiles of [128, NVAR] per group
HC = NVAR // 2            # 288, matmul N-chunk (<=512 per PSUM bank)

_GUARD = 1e-30            # ln guard: avoids ln(0) when te == P == 0
_CLIP_C = float(2.0 * np.arctanh(np.float64(np.float32(1.0 - 1e-6))))
_TE_CAP = float(np.float32(np.tanh(np.float64(np.float32(7.5)))))


def _build(gate: float):
    nc = bacc.Bacc("TRN2", target_bir_lowering=False, debug=False)

    wg_d = nc.dram_tensor("wg", [BL * MCHK, NVAR], F16, kind="ExternalInput").ap()
    offm_d = nc.dram_tensor("offm", [BL * MCHK, NVAR], BF16,
                            kind="ExternalInput").ap()
    xs_d = nc.dram_tensor("xs", [BL, T * NVAR], F32, kind="ExternalInput").ap()
    esel_d = nc.dram_tensor("esel", [128, NT * GI], F16, kind="ExternalInput").ap()
    eselt_d = nc.dram_tensor("eselt", [GI, NT * 128], F32R,
                             kind="ExternalInput").ap()
    posts_d = nc.dram_tensor("posts", [BL, T * NVAR], F32, kind="ExternalOutput").ap()

    one_m_g = float(1.0 - gate)

    with tile.TileContext(nc) as tc:
        with (
            tc.tile_pool(name="consts", bufs=1) as consts,
            tc.tile_pool(name="wg", bufs=2) as wg_pool,
            tc.tile_pool(name="offm", bufs=2) as offm_pool,
            tc.tile_pool(name="mstate", bufs=2) as m_pool,
            tc.tile_pool(name="atile", bufs=3) as a_pool,
            tc.tile_pool(name="vte", bufs=3) as vte_pool,
            tc.tile_pool(name="a12", bufs=2) as a12_pool,
            tc.tile_pool(name="pprod", bufs=2) as p_pool,
            tc.tile_pool(name="psum_v", bufs=2, space="PSUM") as psv_pool,
            tc.tile_pool(name="psum_post", bufs=2, space="PSUM") as psp_pool,
        ):
            esel = consts.tile([128, NT, GI], F16)
            nc.sync.dma_start(out=esel, in_=esel_d.rearrange("p (j g) -> p j g", g=GI))
            eselt = consts.tile([GI, NT, 128], F32R)
            nc.sync.dma_start(
                out=eselt, in_=eselt_d.rearrange("g (j p) -> g j p", p=128)
            )
            b_guard = consts.tile([128, 1], F32)
            nc.vector.memset(b_guard, _GUARD)
            xsall = consts.tile([128, T, 2, HC], F32)
            for g in range(NG):
                nc.sync.dma_start(
                    out=xsall[32 * g : 32 * g + GI],
                    in_=xs_d[g * GI : (g + 1) * GI].rearrange(
                        "b (t c n) -> b t c n", t=T, c=2
                    ),
                )
            postsall = consts.tile([128, T, 2, HC], F32)

            # tile pairs: (0,1) (2,3) (4,5) (6,7) (8,)
            pairs = [(0, 1), (2, 3), (4, 5), (6, 7), (8,)]

            def load_group(g):
                wg_g = wg_pool.tile([128, NT, NVAR], F16)
                nc.sync.dma_start(
                    out=wg_g,
                    in_=wg_d[g * NT * 128 : (g + 1) * NT * 128, :].rearrange(
                        "(j p) n -> p j n", p=128
                    ),
                )
                offm_g = offm_pool.tile([128, NT, NVAR], BF16)
                nc.sync.dma_start(
                    out=offm_g,
                    in_=offm_d[g * NT * 128 : (g + 1) * NT * 128, :].rearrange(
                        "(j p) n -> p j n", p=128
                    ),
                )
                m_g = m_pool.tile([128, NT, NVAR], F32)
                a_cur = a_pool.tile([GI, 2, HC], F32R, tag="a_cur", name="a_cur")
                nc.vector.tensor_copy(a_cur, xsall[32 * g : 32 * g + GI, 0])
                return {"wg": wg_g, "offm": offm_g, "m": m_g, "a": a_cur}

            def emit_group_iter(g, t, st):
                wg_g, offm_g, m_g, a_cur = st["wg"], st["offm"], st["m"], st["a"]
                xs_g = xsall[32 * g : 32 * g + GI]
                if True:
                    post_ps = psp_pool.tile([GI, 2, 512], F32)
                    ptile = p_pool.tile([128, NT], F32, tag="pp", name="pp")
                    a12s = [
                        a12_pool.tile([128, 2, 2, NVAR], BF16, tag="a12",
                                      name="a12")
                        for _ in pairs
                    ]
                    vtes = {}
                    # ---- stage A: broadcast A, subtract M  (PE + DVE) ----
                    for pi, pj in enumerate(pairs):
                        w = len(pj)
                        vte = vte_pool.tile([128, 2, NVAR], F32, tag="vte",
                                            name="vte")[:, :w]
                        vtes[pi] = vte
                        for jj, j in enumerate(pj):
                            v_ps = psv_pool.tile([128, 2, 512], F32)
                            for c in range(2):
                                nc.tensor.matmul(
                                    v_ps[:, c, :HC],
                                    eselt[:, j],
                                    a_cur[:, c],
                                    start=True,
                                    stop=True,
                                )
                            if t == 0:
                                # M = 0: v is just the broadcast A
                                nc.vector.tensor_copy(
                                    vte[:, jj].rearrange("p (c n) -> p c n", c=2),
                                    v_ps[:, :, :HC],
                                )
                            else:
                                nc.vector.tensor_tensor(
                                    out=vte[:, jj].rearrange("p (c n) -> p c n", c=2),
                                    in0=v_ps[:, :, :HC],
                                    in1=m_g[:, j].rearrange("p (c n) -> p c n", c=2),
                                    op=ALU.subtract,
                                )
                    # ---- stage B: tanh (ACT, pair-wide) ----
                    for pi, pj in enumerate(pairs):
                        nc.scalar.activation(vtes[pi], vtes[pi], ACT.Tanh,
                                             bias=0.0, scale=0.5)
                    # ---- stage B2: cap te to tanh(7.5) == reference's V clip ----
                    for pi, pj in enumerate(pairs):
                        nc.vector.tensor_scalar(
                            out=vtes[pi], in0=vtes[pi], scalar1=_TE_CAP,
                            scalar2=-_TE_CAP, op0=ALU.min, op1=ALU.max,
                        )
                    # ---- stage C: row product with off-edge fix ----
                    for pi, pj in enumerate(pairs):
                        w = len(pj)
                        te_p = vtes[pi]
                        nc.vector.tensor_tensor(
                            out=te_p, in0=te_p,
                            in1=offm_g[:, pj[0] : pj[0] + w],
                            op=ALU.max,
                        )
                        for jj, j in enumerate(pj):
                            nc.vector.tensor_reduce(
                                out=ptile[:, j : j + 1],
                                in_=vtes[pi][:, jj],
                                axis=mybir.AxisListType.X,
                                op=ALU.mult,
                            )
                    # ---- stage D: |te +- P|  (ACT abs / DVE ts) ----
                    for pi, pj in enumerate(pairs):
                        for jj, j in enumerate(pj):
                            te = vtes[pi][:, jj]
                            p_t = ptile[:, j : j + 1]
                            nc.scalar.activation(a12s[pi][:, jj, 0], te, ACT.Abs,
                                                 bias=p_t, scale=1.0)
                            nc.scalar.activation(a12s[pi][:, jj, 1], te, ACT.Abs,
                                                 bias=p_t, scale=-1.0)
                    # ---- stage E: ln (ACT, per pair) ----
                    for pi, pj in enumerate(pairs):
                        w = len(pj)
                        nc.scalar.activation(
                            _f16(a12s[pi][:, :w]), a12s[pi][:, :w],
                            ACT.Ln, bias=b_guard,
                        )
                    # ---- stage F..H: lq, d, M update (GPS + DVE) ----
                    for pi, pj in enumerate(pairs):
                        w = len(pj)
                        j0 = pj[0]
                        l1 = _f16(a12s[pi][:, :w, 0])
                        l2 = _f16(a12s[pi][:, :w, 1])
                        nc.gpsimd.tensor_tensor(out=l1, in0=l1, in1=l2,
                                                op=ALU.subtract)
                        nc.vector.tensor_scalar(
                            out=l1, in0=l1, scalar1=_CLIP_C, scalar2=-_CLIP_C,
                            op0=ALU.min, op1=ALU.max,
                        )
                        mpair = m_g[:, j0 : j0 + w]
                        wgp = wg_g[:, j0 : j0 + w]
                        d = _f16(a12s[pi][:, :w, 1])
                        nc.gpsimd.tensor_tensor(out=d, in0=l1, in1=wgp,
                                                op=ALU.mult)
                        if t == 0:
                            nc.vector.tensor_copy(mpair, d)
                        else:
                            nc.vector.scalar_tensor_tensor(
                                out=mpair, in0=mpair, scalar=one_m_g, in1=d,
                                op0=ALU.mult, op1=ALU.add,
                            )
                        # posterior increment: post += Esel_j^T @ d_j  (bf16)
                        for jj, j in enumerate(pj):
                            for c in range(2):
                                nc.tensor.matmul(
                                    post_ps[:, c, :HC],
                                    esel[:, j],
                                    _f16(a12s[pi][:, jj, 1,
                                              c * HC : (c + 1) * HC]),
                                    start=(j == 0),
                                    stop=(j == NT - 1),
                                )
                    # posts(t) = (1-gate)*posts(t-1) + sum_m d  (host adds x_t)
                    posts_t = postsall[32 * g : 32 * g + GI, t]
                    if t == 0:
                        nc.scalar.copy(posts_t, post_ps[:, :, :HC])
                    else:
                        nc.vector.scalar_tensor_tensor(
                            out=posts_t,
                            in0=postsall[32 * g : 32 * g + GI, t - 1],
                            scalar=one_m_g,
                            in1=post_ps[:, :, :HC],
                            op0=ALU.mult,
                            op1=ALU.add,
                        )
                    if t + 1 < T:
                        a_new = a_pool.tile([GI, 2, HC], F32R, tag="a_cur",
                                            name="a_cur")
                        nc.vector.tensor_add(a_new, posts_t, xs_g[:, t + 1])
                        st["a"] = a_new

            for gp in range(0, NG, 2):
                sts = {g: load_group(g) for g in (gp, gp + 1)}
                for t in range(T):
                    for g in (gp, gp + 1):
                        emit_group_iter(g, t, sts[g])

            for g in range(NG):
                nc.sync.dma_start(
                    out=posts_d[g * GI : (g + 1) * GI].rearrange(
                        "b (t c n) -> b t c n", t=T, c=2
                    ),
                    in_=postsall[32 * g : 32 * g + GI],
                )
    nc.compile()
    return nc


_CACHE = {}


def _get_nc(gate: float):
    key = round(gate, 12)
    if key not in _CACHE:
        _CACHE[key] = _build(gate)
    return _CACHE[key]


def _host_prep(inputs, H, sigma2, input_ponderation, w_cv, gate_logit):
    f32 = np.float32
    gate = float(1.0 / (1.0 + np.exp(-np.float64(gate_logit))))

    llrs = (f32(-4.0) * inputs / sigma2).astype(f32)
    norm_llrs = llrs / np.mean(np.abs(llrs), axis=-1, keepdims=True, dtype=f32)
    xs = (norm_llrs[:, None, :] * input_ponderation[None, :, :]).astype(f32)  # [B,T,N]

    Hf = H.astype(f32)
    wg_full = (f32(gate) * w_cv[None, :, :] * Hf).astype(np.float16)
    offm_full = (f32(1.0) - f32(2.0) * Hf).astype(ml_dtypes.bfloat16)  # +1 off, -1 on

    # selector constants (same for every core)
    rows = np.arange(GI * MCHK)
    esel = np.zeros((128, NT, GI), f32)
    eselt = np.zeros((GI, NT, 128), f32)
    for j in range(NT):
        for p in range(128):
            k = int(rows[j * 128 + p] // MCHK)
            esel[p, j, k] = 1.0
            eselt[k, j, p] = 1.0

    in_maps = []
    for c in range(NCORES):
        sl = slice(c * BL, (c + 1) * BL)
        in_maps.append(
            {
                "wg": np.ascontiguousarray(wg_full[sl].reshape(BL * MCHK, NVAR)),
                "offm": np.ascontiguousarray(offm_full[sl].reshape(BL * MCHK, NVAR)),
                "xs": np.ascontiguousarray(xs[sl].reshape(BL, T * NVAR)),
                "esel": np.ascontiguousarray(
                    esel.reshape(128, NT * GI).astype(np.float16)
                ),
                "eselt": np.ascontiguousarray(eselt.reshape(GI, NT * 128)),
            }
        )
    return gate, norm_llrs, xs, in_maps


def _host_post(posts_raw, xs, norm_llrs, out_ponderation, skip_ponderation):
    f32 = np.float32
    posts = (posts_raw + xs).astype(f32)  # add x_t back in
    norm_out = posts / np.mean(np.abs(posts), axis=-1, keepdims=True, dtype=f32)
    pooled = np.mean(out_ponderation[None] * norm_out, axis=-2, dtype=f32)
    out = (pooled + skip_ponderation * norm_llrs).astype(f32)
    return (1.0 / (1.0 + np.exp(out[:, :KINFO], dtype=f32))).astype(f32)


def run(trace=False, **inputs):
    inputs = {k: np.asarray(v) for k, v in inputs.items()}
    gate, norm_llrs, xs, in_maps = _host_prep(
        inputs["inputs"],
        inputs["H"],
        inputs["sigma2"],
        inputs["input_ponderation"],
        inputs["w_cv"],
        inputs["gate_logit"],
    )
    nc = _get_nc(gate)
    res = run_bass_kernel_spmd(
        nc, in_maps, core_ids=list(range(NCORES)), trace=trace
    )
    posts_raw = np.concatenate(
        [r["posts"].reshape(BL, T, NVAR) for r in res.results], axis=0
    )
    out = _host_post(
        posts_raw, xs, norm_llrs,
        inputs["out_ponderation"], inputs["skip_ponderation"],
    )
    return out, res


def kernel(**inputs) -> np.ndarray:
    out, _ = run(trace=False, **inputs)
    return out
